# revision 28
# baseline (speedup 1.0000x reference)
"""AdaptiveBoundaryRankingLoss on 8 TRN2 NeuronCores.

Math: loss = sum_{i<j} relu(boundary(|dt|) - (p_i-p_j)*sign(dt)) / K,
  dt = t_i - t_j, boundary(a) = BETA*a/(1+GAMMA*a), K = B(B-1)/2.

Host sorts (pred,target) by target ascending (the loss is a sum over
unordered pairs, so relabeling is free). After sorting, for i>j
(strict lower triangle) sign(t_i - t_j) = +1, so with
m(a) = a/(1+GAMMA*a), a = t_i - t_j >= 0, dp' = (p_i - p_j)/BETA:
  loss = BETA/K * sum_{i>j} relu(m(a) - dp').

m(a) is approximated per row by a minimax quadratic on a in [0, L_row]
(L_row = t_row - t_min; residual <= max|m_3rd| L^3/192 ~ 3e-4 worst):
  m(a) ~= beta_r - (s_r*(a - a0_r))^2,  q := (s_r*(a - a0_r))^2.

Per 128-row tile k (columns [0, W), W = (k+1)*1024):
  q: one ScalarE Square activation per tile (per-row scale/bias APs)
     for tiles 0,3,4,5,6 and the lower half of tile 7; tiles 1-2 are
     computed by VectorE (2-op tensor_scalar + x*x tensor_tensor) in
     its early idle window; tile 7's upper half is host-baked (qv7).
  z = pc' - q   (tensor_tensor sub; the 1024-wide diagonal edge window
     reads a host-baked edge_pc whose invalid columns are -30000)
  w = relu(z + (beta_r - pr'))   (one 2-op tensor_scalar, 4x mode)
  TensorE: psum[1,512] += ones[128,1]^T @ w[:,chunk]  (column sums)
ScalarE drains the PSUM row to SBUF at the end; host computes
loss = BETA * sum(out) / K.

DMA is split across two queues with independent semaphores (sync:
ScalarE-side bytes, i.e. tables + tcneg column chunks; gpsimd:
VectorE-side bytes, i.e. pcb2/edge_pc chunks + qv7) so each engine
gates only on the bytes it actually reads and compute starts after
~0.5MB instead of 6MB. Tile 0's Square and z are split at column 512
to start earlier; tile 7's relu runs in four 2048-column phases so
TensorE's final matmuls overlap the last relu.

Work split: 64 row-blocks of 128 rows; core c takes blocks {8k+c},
tile k spans columns [0,(k+1)*1024) -> identical graph on all cores
(SPMD); per-core differences live in input data (tables + edge_pc +
qv7). The kernel executes the NEFF twice and returns the second
(warmed) result.
"""

import contextlib

import numpy as np
import ml_dtypes

import concourse.bass as bass
from concourse import mybir
from concourse.bass_utils import run_bass_kernel_spmd

B = 8192
BETA = 0.3
GAMMA = 0.1
NCORES = 8
NT = 8          # tiles per core
TW = 1024       # column granularity; tile k has W_k = (k+1)*TW columns
P = 128
MMW = 512       # matmul chunk width (one PSUM bank of f32)
VQ0 = 4 * TW    # tile 7 columns [VQ0, B) compute q on VectorE
TAIL = 2 * TW   # tile 7 phase-2 width (TensorE tail shortening)

_bf16 = ml_dtypes.bfloat16

_NC_CACHE = None


def build_nc():
    nc = bass.Bass(target_bir_lowering=False, debug=False)
    f32 = mybir.dt.float32
    bf16 = mybir.dt.bfloat16
    A = mybir.AluOpType

    tcneg_d = nc.declare_dram_parameter("tcneg", [P, B], bf16, isOutput=False)
    pcb2_d = nc.declare_dram_parameter("pcb2", [P, B], bf16, isOutput=False)
    edge_d = nc.declare_dram_parameter("edgepc", [P, NT * TW], bf16, isOutput=False)
    sqs_d = nc.declare_dram_parameter("sqs", [P, NT], f32, isOutput=False)
    sqb_d = nc.declare_dram_parameter("sqb", [P, NT], f32, isOutput=False)
    ubt_d = nc.declare_dram_parameter("ubt", [P, NT], f32, isOutput=False)
    ones_d = nc.declare_dram_parameter("ones", [P, 1], bf16, isOutput=False)
    qv7_d = nc.declare_dram_parameter("qv7", [P, B - VQ0], bf16, isOutput=False)
    out_d = nc.declare_dram_parameter("out", [1, MMW], f32, isOutput=True)

    es = contextlib.ExitStack()
    with es:
        def sb(name, shape, dtype):
            return es.enter_context(nc.sbuf_tensor(name, shape, dtype))

        tcneg = sb("tcneg_s", [P, B], bf16)
        pcb2 = sb("pcb2_s", [P, B], bf16)
        edgepc = sb("edgepc_s", [P, NT * TW], bf16)
        sqs = sb("sqs_s", [P, NT], f32)
        sqb = sb("sqb_s", [P, NT], f32)
        ubt = sb("ubt_s", [P, NT], f32)
        ones = sb("ones_s", [P, 1], bf16)
        q0 = sb("q0", [P, B], bf16)
        q1 = sb("q1", [P, B], bf16)
        q2 = sb("q2", [P, B], bf16)
        q3 = sb("q3", [P, B], bf16)
        qv = sb("qv", [P, B - VQ0], bf16)
        xb = sb("xb", [P, 3 * TW], bf16)
        zb = sb("zb", [P, B], bf16)
        w0 = sb("w0", [P, B], bf16)
        w1 = sb("w1", [P, B], bf16)
        osb = sb("osb", [1, MMW], f32)
        ps = es.enter_context(nc.psum_tensor("ps", [1, MMW], f32))
        dma_a = es.enter_context(nc.semaphore("dma_a"))
        dma_b = es.enter_context(nc.semaphore("dma_b"))
        se_sem = es.enter_context(nc.semaphore("se_sem"))
        ve_sem = es.enter_context(nc.semaphore("ve_sem"))
        te_sem = es.enter_context(nc.semaphore("te_sem"))
        block = es.enter_context(nc.Block())

        qbufs = [q0, q1, q2, q3]
        wbufs = [w0, w1]

        @block.sync
        def _(sync):
            # queue a: ScalarE-side bytes (+ ones for TensorE)
            for dst, src in [
                (sqs[:, :], sqs_d[:, :]),                       # a>=16
                (sqb[:, :], sqb_d[:, :]),                       # a>=32
                (tcneg[:, :512], tcneg_d[:, :512]),             # a>=48
                (tcneg[:, 512:TW], tcneg_d[:, 512:TW]),         # a>=64
                (ones[:, :], ones_d[:, :]),                     # a>=80
                (tcneg[:, TW:2 * TW], tcneg_d[:, TW:2 * TW]),   # a>=96
                (tcneg[:, 2 * TW:4 * TW], tcneg_d[:, 2 * TW:4 * TW]),  # a>=112
                (tcneg[:, 4 * TW:], tcneg_d[:, 4 * TW:]),       # a>=128
            ]:
                sync.dma_start(out=dst, in_=src).then_inc(dma_a, 16)
            sync.wait_ge(se_sem, 9)
            sync.dma_start(out=out_d[:, :], in_=osb[:, :]).then_inc(dma_a, 16)

        @block.gpsimd
        def _(gpsimd):
            # queue b: VectorE-side bytes
            for dst, src in [
                (ubt[:, :], ubt_d[:, :]),                               # b>=16
                (pcb2[:, :512], pcb2_d[:, :512]),                       # b>=32
                (edgepc[:, :512], edge_d[:, :512]),                     # b>=48
                (pcb2[:, 512:TW], pcb2_d[:, 512:TW]),                   # b>=64
                (edgepc[:, 512:TW], edge_d[:, 512:TW]),                 # b>=80
                (edgepc[:, TW:2 * TW], edge_d[:, TW:2 * TW]),           # b>=96
                (pcb2[:, TW:2 * TW], pcb2_d[:, TW:2 * TW]),             # b>=112
                (edgepc[:, 2 * TW:4 * TW], edge_d[:, 2 * TW:4 * TW]),   # b>=128
                (pcb2[:, 2 * TW:4 * TW], pcb2_d[:, 2 * TW:4 * TW]),     # b>=144
                (pcb2[:, 4 * TW:], pcb2_d[:, 4 * TW:]),                 # b>=160
                (edgepc[:, 4 * TW:], edge_d[:, 4 * TW:]),               # b>=176
                (qv[:, :], qv7_d[:, :]),                                # b>=192
            ]:
                gpsimd.dma_start(out=dst, in_=src).then_inc(dma_b, 16)

        @block.scalar
        def _(scalar):
            # dummy 1-element Square: pulls ACT_TABLE_LOAD to t=0
            scalar.activation(
                q0[:, 0:1], q0[:, 0:1],
                mybir.ActivationFunctionType.Square,
            )
            Sq = mybir.ActivationFunctionType.Square
            # q0 in two halves so the first Square starts on 512 columns
            scalar.wait_ge(dma_a, 48)
            scalar.activation(q0[:, 0:512], tcneg[:, 0:512], Sq,
                              bias=sqb[:, 0:1], scale=sqs[:, 0:1],
                              ).then_inc(se_sem, 1)
            scalar.wait_ge(dma_a, 64)
            scalar.activation(q0[:, 512:TW], tcneg[:, 512:TW], Sq,
                              bias=sqb[:, 0:1], scale=sqs[:, 0:1],
                              ).then_inc(se_sem, 1)
            for k in range(3, NT):
                W = (k + 1) * TW if k < NT - 1 else VQ0
                if k == 3:
                    scalar.wait_ge(dma_a, 112)
                if k == 4:
                    scalar.wait_ge(dma_a, 128)
                if k >= 4:
                    # q buffer reuse (4-deep): wait VE consumed tile k-4
                    scalar.wait_ge(ve_sem, k - 3)
                if k < NT - 1:
                    # q = (s_r*(a-a0_r))^2 ; in = -tc, per-row scale/bias
                    scalar.activation(
                        qbufs[k % 4][:, :W], tcneg[:, :W], Sq,
                        bias=sqb[:, k:k + 1], scale=sqs[:, k:k + 1],
                    ).then_inc(se_sem, 1)
                else:
                    # tile 7 lower half, split so VE starts earlier
                    for lo, hi in [(0, VQ0 // 2), (VQ0 // 2, VQ0)]:
                        scalar.activation(
                            qbufs[k % 4][:, lo:hi], tcneg[:, lo:hi], Sq,
                            bias=sqb[:, k:k + 1], scale=sqs[:, k:k + 1],
                        ).then_inc(se_sem, 1)
            # drain PSUM to SBUF once TensorE is done
            scalar.wait_ge(te_sem, NT)
            scalar.copy(osb[:, :], ps[0:1, :]).then_inc(se_sem, 1)

        @block.vector
        def _(vector):
            vector.wait_ge(dma_b, 48)
            vector.wait_ge(se_sem, 1)
            # tile 0 (all edge window), split to start early
            vector.tensor_tensor(out=zb[:, 0:512], in0=edgepc[:, 0:512],
                                 in1=q0[:, 0:512], op=A.subtract)
            vector.wait_ge(dma_b, 80)
            vector.wait_ge(se_sem, 2)
            vector.tensor_tensor(out=zb[:, 512:TW], in0=edgepc[:, 512:TW],
                                 in1=q0[:, 512:TW], op=A.subtract)
            vector.tensor_scalar(
                out=w0[:, :TW], in0=zb[:, :TW],
                scalar1=ubt[:, 0:1], scalar2=0.0, op0=A.add, op1=A.max,
            ).then_inc(ve_sem, 1)
            for k in range(1, NT):
                W = (k + 1) * TW
                if k == 1:
                    vector.wait_ge(dma_b, 96)
                    vector.wait_ge(dma_a, 96)
                if k == 2:
                    vector.wait_ge(dma_b, 128)
                    vector.wait_ge(dma_a, 112)
                if k == 3:
                    vector.wait_ge(dma_b, 144)
                if k == 4:
                    vector.wait_ge(dma_b, 176)
                if k == NT - 1:
                    vector.wait_ge(dma_b, 192)
                if k in (1, 2):
                    # self-q: x = sqs*(-tc)+sqb ; q = x*x
                    vector.tensor_scalar(
                        out=xb[:, :W], in0=tcneg[:, :W],
                        scalar1=sqs[:, k:k + 1], scalar2=sqb[:, k:k + 1],
                        op0=A.mult, op1=A.add,
                    )
                    vector.tensor_tensor(out=qbufs[k][:, :W], in0=xb[:, :W],
                                         in1=xb[:, :W], op=A.mult)
                elif k < NT - 1:
                    vector.wait_ge(se_sem, k)
                if k < NT - 1:
                    phases = [(0, W)]
                else:
                    phases = [(0, VQ0 // 2), (VQ0 // 2, VQ0),
                              (VQ0, 6 * TW), (6 * TW, W)]
                q = qbufs[k % 4]
                for pi, (lo, hi) in enumerate(phases):
                    if k == NT - 1 and pi < 2:
                        vector.wait_ge(se_sem, 7 + pi)
                    # z = pc' - q (main region), edge window uses edge_pc
                    mhi = min(hi, k * TW)
                    ranges = [(lo, mhi, pcb2)]
                    elo = max(lo, k * TW)
                    if hi > elo:
                        ranges.append((elo, hi, edgepc))
                    for zlo, zhi, srct in ranges:
                        if zhi <= zlo:
                            continue
                        if k == NT - 1 and zlo >= VQ0:
                            qa = qv[:, zlo - VQ0:zhi - VQ0]
                        else:
                            qa = q[:, zlo:zhi]
                        vector.tensor_tensor(
                            out=zb[:, zlo:zhi], in0=srct[:, zlo:zhi],
                            in1=qa, op=A.subtract,
                        )
                    if k >= 2 and lo == 0:
                        vector.wait_ge(te_sem, k - 1)
                    # w = relu(z + (beta_r - pr'))
                    vector.tensor_scalar(
                        out=wbufs[k % 2][:, lo:hi], in0=zb[:, lo:hi],
                        scalar1=ubt[:, k:k + 1], scalar2=0.0,
                        op0=A.add, op1=A.max,
                    ).then_inc(ve_sem, 1)

        @block.tensor
        def _(tensor):
            tensor.wait_ge(dma_a, 80)
            for k in range(NT):
                W = (k + 1) * TW
                tensor.wait_ge(ve_sem, k + 1)
                for c in range(W // MMW):
                    if k == NT - 1 and c >= 4 and c % 4 == 0:
                        tensor.wait_ge(ve_sem, 8 + c // 4)
                    mm = tensor.matmul(
                        ps[:, :], ones[:, :],
                        wbufs[k % 2][:, c * MMW:(c + 1) * MMW],
                        start=(k == 0 and c == 0),
                        stop=(k == NT - 1 and c == (W // MMW) - 1),
                    )
                mm.then_inc(te_sem, 1)

    return nc


def _get_nc():
    global _NC_CACHE
    if _NC_CACHE is None:
        _NC_CACHE = build_nc()
    return _NC_CACHE


def _quad_fit_rows(L, n=48):
    """Vectorized per-row quadratic fit of m(a)=a/(1+G*a) on [0, L_r]
    via Chebyshev interpolation (degree 2). Returns coeff arrays
    (c0, c1, c2) of p(a) = c0 + c1*a + c2*a^2."""
    L = np.maximum(np.asarray(L, np.float64), 1e-3)
    n_ = n
    xk = np.cos((2 * np.arange(n_) + 1) * np.pi / (2 * n_))
    a = (xk[None, :] + 1.0) * 0.5 * L[:, None]          # [rows, n]
    f = a / (1.0 + GAMMA * a)
    b0 = f @ (np.ones_like(xk) / n_)
    b1 = f @ (xk * 2.0 / n_)
    b2 = f @ ((2 * xk * xk - 1.0) * 2.0 / n_)
    # p(x) = (b0 - b2) + b1*x + 2*b2*x^2,  x = 2a/L - 1
    A0 = b0 - b2
    A1 = b1
    A2 = 2 * b2
    c0 = A0 - A1 + A2
    c1 = (A1 - 2 * A2) * 2.0 / L
    c2 = A2 * 4.0 / (L * L)
    return c0, c1, c2


def _make_in_maps(pred, target):
    order = np.argsort(target, kind="stable")
    t = target[order].astype(np.float64)
    p = pred[order].astype(np.float64)
    tmin = t[0]

    tcneg_1d = (-t).astype(_bf16)
    pcb2_1d = (p / BETA).astype(_bf16)
    tcneg_full = np.ascontiguousarray(np.broadcast_to(tcneg_1d[None, :], (P, B)))
    pcb2_full = np.ascontiguousarray(np.broadcast_to(pcb2_1d[None, :], (P, B)))
    ones = np.ones((P, 1), dtype=_bf16)

    jw = np.arange(TW)[None, :]
    pp = np.arange(P)[:, None]

    in_maps = []
    for c in range(NCORES):
        # rows[p, k] = global row of partition p in tile k
        rows = (8 * np.arange(NT)[None, :] + c) * P + pp
        tr = t[rows]                       # [128, 8]
        pr = p[rows] / BETA
        c0, c1, c2 = _quad_fit_rows((tr - tmin).ravel())
        c0 = c0.reshape(P, NT)
        c1 = c1.reshape(P, NT)
        c2 = np.minimum(c2.reshape(P, NT), -1e-8)
        s = np.sqrt(-c2)
        a0 = -c1 / (2 * c2)
        beta_r = c0 - c2 * a0 * a0
        # edge_pc[p, k*TW + jw] = valid ? pcb2[k*TW+jw] : -30000
        valid = (jw < c * P + pp)          # [128, TW], same for every k
        edge = np.empty((P, NT * TW), dtype=_bf16)
        for k in range(NT):
            vals = np.broadcast_to(pcb2_1d[k * TW:(k + 1) * TW][None, :], (P, TW))
            edge[:, k * TW:(k + 1) * TW] = np.where(
                valid, vals, _bf16(-30000.0))
        sqs7 = s[:, NT - 1:NT].astype(np.float32).astype(np.float64)
        sqb7 = (s * (tr - a0))[:, NT - 1:NT].astype(np.float32).astype(np.float64)
        tcb = tcneg_1d.astype(np.float64)[VQ0:]   # bf16-rounded -tc
        qv7 = ((sqs7 * tcb[None, :] + sqb7) ** 2).astype(_bf16)
        in_maps.append({
            "tcneg": tcneg_full,
            "pcb2": pcb2_full,
            "qv7": qv7,
            "edgepc": edge,
            "sqs": s.astype(np.float32),
            "sqb": (s * (tr - a0)).astype(np.float32),
            "ubt": (beta_r - pr).astype(np.float32),
            "ones": ones,
        })
    return in_maps


def kernel(pred, target):
    pred = np.asarray(pred, dtype=np.float32)
    target = np.asarray(target, dtype=np.float32)
    in_maps = _make_in_maps(pred, target)
    nc = _get_nc()
    run_bass_kernel_spmd(nc, in_maps, core_ids=list(range(NCORES)))
    res = run_bass_kernel_spmd(nc, in_maps, core_ids=list(range(NCORES)))
    total = 0.0
    for r in res.results:
        total += np.asarray(r["out"], dtype=np.float64).sum()
    K = B * (B - 1) // 2
    return np.float32(BETA * total / K)
